# revision 2
# baseline (speedup 1.0000x reference)
import numpy as np
import jax
import jax.numpy as jnp
from jax import lax
from functools import partial

NEG_SLOPE = 0.1
FPN = 256
B = 4
SIZES = [(128, 96), (64, 48), (32, 24), (16, 12), (8, 6)]


def _leaky(x):
    return jnp.where(x >= 0, x, NEG_SLOPE * x)


def _conv3x3(x, w, b):
    y = lax.conv_general_dilated(x, w, (1, 1), ((1, 1), (1, 1)),
                                 dimension_numbers=('NCHW', 'OIHW', 'NCHW'))
    return y + b[None, :, None, None]


def _conv_stack(x, params):
    n = len(params)
    for j, (w, b) in enumerate(params):
        x = _conv3x3(x, w, b)
        if j < n - 1:
            x = _leaky(x)
    return x


def _bilinear_grid_sample(im, grid):
    n, c, h, w = im.shape
    x = ((grid[..., 0] + 1) * w - 1) / 2
    y = ((grid[..., 1] + 1) * h - 1) / 2
    x0 = jnp.floor(x); y0 = jnp.floor(y)
    x1 = x0 + 1; y1 = y0 + 1
    wa = ((x1 - x) * (y1 - y))[:, None]
    wb = ((x1 - x) * (y - y0))[:, None]
    wc = ((x - x0) * (y1 - y))[:, None]
    wd = ((x - x0) * (y - y0))[:, None]
    imp = jnp.pad(im, ((0, 0), (0, 0), (1, 1), (1, 1)), mode='edge')
    ph, pw = h + 2, w + 2
    xi0 = jnp.clip(x0.astype(jnp.int32) + 1, 0, pw - 1)
    xi1 = jnp.clip(x1.astype(jnp.int32) + 1, 0, pw - 1)
    yi0 = jnp.clip(y0.astype(jnp.int32) + 1, 0, ph - 1)
    yi1 = jnp.clip(y1.astype(jnp.int32) + 1, 0, ph - 1)
    gather = jax.vmap(lambda img, yy, xx: img[:, yy, xx])
    Ia = gather(imp, yi0, xi0)
    Ib = gather(imp, yi1, xi0)
    Ic = gather(imp, yi0, xi1)
    Id = gather(imp, yi1, xi1)
    return Ia * wa + Ib * wb + Ic * wc + Id * wd


def _apply_offset(offset):
    n, _, h, w = offset.shape
    gy, gx = jnp.meshgrid(jnp.arange(h, dtype=offset.dtype),
                          jnp.arange(w, dtype=offset.dtype), indexing='ij')
    xg = gx[None] + offset[:, 0]
    yg = gy[None] + offset[:, 1]
    xg = xg / ((w - 1.0) / 2.0) - 1.0
    yg = yg / ((h - 1.0) / 2.0) - 1.0
    return jnp.stack([xg, yg], axis=-1)


def _corr(a, b, max_disp=3, dila=1):
    n, c, h, w = a.shape
    pad = max_disp * dila
    bp = jnp.pad(b, ((0, 0), (0, 0), (pad, pad), (pad, pad)))
    outs = []
    for dy in range(2 * max_disp + 1):
        for dx in range(2 * max_disp + 1):
            sh = bp[:, :, dy * dila:dy * dila + h, dx * dila:dx * dila + w]
            outs.append(jnp.mean(a * sh, axis=1))
    return jnp.stack(outs, axis=1)


def _upsample2x(x):
    n, c, h, w = x.shape
    return jax.image.resize(x, (n, c, 2 * h, 2 * w), method='bilinear')


def _unflatten_params(flat):
    # flat: list of 8 arrays (w0,b0,...,w3,b3)
    return [(flat[2 * j], flat[2 * j + 1]) for j in range(4)]


def _level_first(xw, xc, mp0, mp1, mp2, mp3, mp4, mp5, mp6, mp7,
                 rp0, rp1, rp2, rp3, rp4, rp5, rp6, rp7):
    main_params = _unflatten_params([mp0, mp1, mp2, mp3, mp4, mp5, mp6, mp7])
    refine_params = _unflatten_params([rp0, rp1, rp2, rp3, rp4, rp5, rp6, rp7])
    ten_corr = _leaky(_corr(xw, xc, max_disp=3, dila=1))
    flow = _conv_stack(ten_corr, main_params)
    flow = _apply_offset(flow)
    flow = jnp.transpose(flow, (0, 3, 1, 2))
    last_flow = flow
    xw2 = _bilinear_grid_sample(xw, jnp.transpose(flow, (0, 2, 3, 1)))
    flow = _conv_stack(jnp.concatenate([xw2, xc], axis=1), refine_params)
    flow = _apply_offset(flow)
    flow = _bilinear_grid_sample(last_flow, flow)
    return _upsample2x(flow)


def _level_rest(xw, xc, last_flow, mp0, mp1, mp2, mp3, mp4, mp5, mp6, mp7,
                rp0, rp1, rp2, rp3, rp4, rp5, rp6, rp7):
    main_params = _unflatten_params([mp0, mp1, mp2, mp3, mp4, mp5, mp6, mp7])
    refine_params = _unflatten_params([rp0, rp1, rp2, rp3, rp4, rp5, rp6, rp7])
    xw_after = _bilinear_grid_sample(
        xw, lax.stop_gradient(jnp.transpose(last_flow, (0, 2, 3, 1))))
    ten_corr = _leaky(_corr(xw_after, xc, max_disp=3, dila=1))
    flow = _conv_stack(ten_corr, main_params)
    flow = _apply_offset(flow)
    flow = _bilinear_grid_sample(last_flow, flow)
    last_flow = flow
    xw2 = _bilinear_grid_sample(xw, jnp.transpose(flow, (0, 2, 3, 1)))
    flow = _conv_stack(jnp.concatenate([xw2, xc], axis=1), refine_params)
    flow = _apply_offset(flow)
    flow = _bilinear_grid_sample(last_flow, flow)
    return _upsample2x(flow)


def _final(x, last_flow):
    x_warp = _bilinear_grid_sample(x, jnp.transpose(last_flow, (0, 2, 3, 1)))
    return x_warp, last_flow


_p_first = jax.pmap(_level_first)
_p_rest = jax.pmap(_level_rest)
_p_final = jax.pmap(_final)


def kernel(x, x_warp0, x_warp1, x_warp2, x_warp3, x_warp4,
           x_cond0, x_cond1, x_cond2, x_cond3, x_cond4,
           main_params, refine_params):
    """Data-parallel AFlowNet on TRN2 NeuronCores: batch (4) sharded one
    element per core via pmap, conv weights replicated; levels run as
    separate on-device modules with flows staying device-resident."""
    x = np.asarray(x)
    warps = [np.asarray(a) for a in
             (x_warp0, x_warp1, x_warp2, x_warp3, x_warp4)]
    conds = [np.asarray(a) for a in
             (x_cond0, x_cond1, x_cond2, x_cond3, x_cond4)]
    nb = x.shape[0]

    def shard(a):
        return a.reshape(nb, 1, *a.shape[1:])

    def repl(a):
        a = np.asarray(a)
        return np.broadcast_to(a[None], (nb, *a.shape))

    lf = None
    for i in range(5):
        xw = shard(warps[-1 - i])
        xc = shard(conds[-1 - i])
        mp = [repl(t) for (w_, b_) in main_params[i] for t in (w_, b_)]
        rp = [repl(t) for (w_, b_) in refine_params[i] for t in (w_, b_)]
        if lf is None:
            lf = _p_first(xw, xc, *mp, *rp)
        else:
            lf = _p_rest(xw, xc, lf, *mp, *rp)
    xw_out, lf_out = _p_final(shard(x), lf)
    xw_out = np.asarray(xw_out).reshape(nb, *xw_out.shape[2:])
    lf_out = np.asarray(lf_out).reshape(nb, *lf_out.shape[2:])
    return xw_out, lf_out


# revision 5
# speedup vs baseline: 14.5778x; 14.5778x over previous
import os
import numpy as np
import jax
import jax.numpy as jnp
from jax import lax

_CACHE_DIR = os.environ.get("KERNEL_JAX_CACHE", "/tmp/aflownet_jax_cache")
try:
    os.makedirs(_CACHE_DIR, exist_ok=True)
    jax.config.update("jax_compilation_cache_dir", _CACHE_DIR)
    jax.config.update("jax_persistent_cache_min_compile_time_secs", 0.5)
except Exception:
    pass

NEG_SLOPE = 0.1
BF = jnp.float32   # compute dtype for features (bf16 overflows the 2e-2 gate)
F32 = jnp.float32


def _leaky(x):
    return jnp.where(x >= 0, x, jnp.asarray(NEG_SLOPE, x.dtype) * x)


def _conv3x3(x, w, b):
    y = lax.conv_general_dilated(x, w.astype(x.dtype), (1, 1), ((1, 1), (1, 1)),
                                 dimension_numbers=('NCHW', 'OIHW', 'NCHW'),
                                 preferred_element_type=F32)
    return y + b[None, :, None, None]


def _conv_stack(x, params):
    # x: bf16; accumulate f32, re-quantize between layers; final out f32
    n = len(params)
    for j, (w, b) in enumerate(params):
        x = _conv3x3(x, w, b)
        if j < n - 1:
            x = _leaky(x).astype(BF)
    return x  # f32


def _bilinear_grid_sample(im, grid):
    # im: any dtype (bf16 for features), grid: f32; returns f32-weighted mix
    n, c, h, w = im.shape
    x = ((grid[..., 0] + 1) * w - 1) / 2
    y = ((grid[..., 1] + 1) * h - 1) / 2
    x0 = jnp.floor(x); y0 = jnp.floor(y)
    x1 = x0 + 1; y1 = y0 + 1
    wa = ((x1 - x) * (y1 - y))[:, None]
    wb = ((x1 - x) * (y - y0))[:, None]
    wc = ((x - x0) * (y1 - y))[:, None]
    wd = ((x - x0) * (y - y0))[:, None]
    imp = jnp.pad(im, ((0, 0), (0, 0), (1, 1), (1, 1)), mode='edge')
    ph, pw = h + 2, w + 2
    xi0 = jnp.clip(x0.astype(jnp.int32) + 1, 0, pw - 1)
    xi1 = jnp.clip(x1.astype(jnp.int32) + 1, 0, pw - 1)
    yi0 = jnp.clip(y0.astype(jnp.int32) + 1, 0, ph - 1)
    yi1 = jnp.clip(y1.astype(jnp.int32) + 1, 0, ph - 1)
    impf = imp.reshape(n, c, ph * pw)
    f00 = (yi0 * pw + xi0).reshape(n, -1)
    f01 = (yi0 * pw + xi1).reshape(n, -1)
    f10 = (yi1 * pw + xi0).reshape(n, -1)
    f11 = (yi1 * pw + xi1).reshape(n, -1)
    gat = jax.vmap(lambda img, ii: jnp.take(img, ii, axis=1))
    sh = x.shape
    Ia = gat(impf, f00).reshape(n, c, *sh[1:])
    Ic = gat(impf, f01).reshape(n, c, *sh[1:])
    Ib = gat(impf, f10).reshape(n, c, *sh[1:])
    Id = gat(impf, f11).reshape(n, c, *sh[1:])
    return Ia * wa + Ib * wb + Ic * wc + Id * wd


def _apply_offset(offset):
    n, _, h, w = offset.shape
    gy, gx = jnp.meshgrid(jnp.arange(h, dtype=offset.dtype),
                          jnp.arange(w, dtype=offset.dtype), indexing='ij')
    xg = gx[None] + offset[:, 0]
    yg = gy[None] + offset[:, 1]
    xg = xg / ((w - 1.0) / 2.0) - 1.0
    yg = yg / ((h - 1.0) / 2.0) - 1.0
    return jnp.stack([xg, yg], axis=-1)


def _corr(a, b, max_disp=3):
    n, c, h, w = a.shape
    pad = max_disp
    bp = jnp.pad(b, ((0, 0), (0, 0), (pad, pad), (pad, pad)))
    outs = []
    for dy in range(2 * max_disp + 1):
        for dx in range(2 * max_disp + 1):
            sh = bp[:, :, dy:dy + h, dx:dx + w]
            outs.append(jnp.mean((a * sh).astype(F32), axis=1))
    return jnp.stack(outs, axis=1)


def _upsample2x(x):
    n, c, h, w = x.shape
    return jax.image.resize(x, (n, c, 2 * h, 2 * w), method='bilinear')


def _level(xw, xc, lf, mp, rp):
    # xw, xc bf16 features; lf f32 flow or None
    if lf is not None:
        xw_after = _bilinear_grid_sample(
            xw, jnp.transpose(lf, (0, 2, 3, 1))).astype(BF)
    else:
        xw_after = xw
    ten_corr = _leaky(_corr(xw_after, xc)).astype(BF)
    flow = _conv_stack(ten_corr, mp)
    flow = _apply_offset(flow)
    if lf is not None:
        flow = _bilinear_grid_sample(lf, flow)
    else:
        flow = jnp.transpose(flow, (0, 3, 1, 2))
    lf2 = flow
    xw2 = _bilinear_grid_sample(xw, jnp.transpose(flow, (0, 2, 3, 1))).astype(BF)
    flow = _conv_stack(jnp.concatenate([xw2, xc], axis=1), rp)
    flow = _apply_offset(flow)
    flow = _bilinear_grid_sample(lf2, flow)
    return _upsample2x(flow)


def _stage_small(xws, xcs, mps, rps):
    # levels i=0,1,2 (sizes (8,6),(16,12),(32,24)) in one module
    lf = None
    for i in range(3):
        lf = _level(xws[i].astype(BF), xcs[i].astype(BF), lf, mps[i], rps[i])
    return lf


def _stage_mid(xw, xc, lf, mp, rp):      # level 3: (64,48)
    return _level(xw.astype(BF), xc.astype(BF), lf, mp, rp)


def _stage_big(xw, xc, lf, mp, rp):      # level 4: (128,96)
    return _level(xw.astype(BF), xc.astype(BF), lf, mp, rp)


def _stage_final(x, lf):
    xwarp = _bilinear_grid_sample(
        x.astype(BF), jnp.transpose(lf, (0, 2, 3, 1))).astype(F32)
    return xwarp, lf


_p_small = jax.pmap(_stage_small)
_p_mid = jax.pmap(_stage_mid)
_p_big = jax.pmap(_stage_big)
_p_final = jax.pmap(_stage_final)


def prepare(x, x_warp0, x_warp1, x_warp2, x_warp3, x_warp4,
            x_cond0, x_cond1, x_cond2, x_cond3, x_cond4,
            main_params, refine_params):
    """Shard inputs batch-wise (one element per NeuronCore) and move to
    device. Returns the argument pytree for run()."""
    x = np.asarray(x)
    warps = [np.asarray(a) for a in
             (x_warp0, x_warp1, x_warp2, x_warp3, x_warp4)]
    conds = [np.asarray(a) for a in
             (x_cond0, x_cond1, x_cond2, x_cond3, x_cond4)]
    nb = x.shape[0]
    devs = jax.devices()[:nb]

    def shard(a):
        return jax.device_put_sharded([a[i:i + 1] for i in range(nb)], devs)

    def repl(a):
        a = np.asarray(a)
        return jax.device_put_replicated(a, devs)

    mps = [[(repl(w), repl(b)) for (w, b) in lvl] for lvl in main_params]
    rps = [[(repl(w), repl(b)) for (w, b) in lvl] for lvl in refine_params]
    args = dict(
        x=shard(x),
        xws_small=[shard(warps[4 - i]) for i in range(3)],
        xcs_small=[shard(conds[4 - i]) for i in range(3)],
        xw_mid=shard(warps[1]), xc_mid=shard(conds[1]),
        xw_big=shard(warps[0]), xc_big=shard(conds[0]),
        mps=mps, rps=rps, nb=nb,
    )
    return args


def run(a):
    lf = _p_small(a["xws_small"], a["xcs_small"], a["mps"][:3], a["rps"][:3])
    lf = _p_mid(a["xw_mid"], a["xc_mid"], lf, a["mps"][3], a["rps"][3])
    lf = _p_big(a["xw_big"], a["xc_big"], lf, a["mps"][4], a["rps"][4])
    xw_out, lf_out = _p_final(a["x"], lf)
    return xw_out, lf_out


def kernel(x, x_warp0, x_warp1, x_warp2, x_warp3, x_warp4,
           x_cond0, x_cond1, x_cond2, x_cond3, x_cond4,
           main_params, refine_params):
    """Data-parallel AFlowNet on the 8-core TRN2 chip: batch dim (4) is
    sharded one element per NeuronCore (pmap), conv weights replicated.
    The pyramid runs as 4 on-device modules with flows device-resident."""
    a = prepare(x, x_warp0, x_warp1, x_warp2, x_warp3, x_warp4,
                x_cond0, x_cond1, x_cond2, x_cond3, x_cond4,
                main_params, refine_params)
    xw_out, lf_out = run(a)
    nb = a["nb"]
    xw_out = np.asarray(xw_out).reshape(nb, *xw_out.shape[2:])
    lf_out = np.asarray(lf_out).reshape(nb, *lf_out.shape[2:])
    return xw_out, lf_out
